# revision 65
# baseline (speedup 1.0000x reference)
"""Bahdanau attention Trainium2 kernel, v2.

Math: out = softmax_k(mask(score)) @ values with
  score[b,q,k] = sum_h wv[h] * tanh(Q[b,q,h] + K[b,k,h]),
  Q = queries @ wq, K = keys @ wk.

tanh(x) ~= sum_m alpha_m sin(omega_m x), omega_m = (m+1) pi / L; the
angle-addition identity factorizes tanh(q+k) into 2M dense [Tq,H]x[H,Tk]
matmuls.  v2 balances trig generation across all five engines:

 - Fixed-point phases (x * 65536/(2L) as int32; ACT reads the low 16
   bits via a bitcast stride-2 view = phase mod 2pi; int16 DVE ALU ops
   saturate on this HW so the arithmetic stays int32).  The fundamental
   comes off the projection PSUM with one convert per side; harmonic m's
   phases are (m+1)*n1 (+16384 quarter-turn for cos), two 2x DVE ops;
   the two m0 cos offsets run on the otherwise-idle Pool engine.
 - k-side trig: ACT Sin per harmonic for the low harmonics; the two
   truncated top harmonics come from one stride-2 Chebyshev step each
   (sin((j+2)w x) = 2 cos(2 w x) sin(j w x) - sin((j-2) w x)) on DVE,
   with exact ACT-computed parents, so the Exp table reload starts two
   Sins earlier.
 - q-side trig: ACT Sin for the fundamental only, then a bf16 Chebyshev
   ladder on DVE (2 tensor_tensor ops per harmonic, 2x mode; the 2*cos
   multiplier is read with a stride-0 quadrature-broadcast AP).
 - alpha_m * wv fold: per-partition-pointer tensor_scalar on the smaller
   q side; Pool engine for all but the exp-gating last harmonic.  The
   host sorts h-channels by |wv| descending (an exact permutation), and
   the two smallest-alpha harmonics use only the heavy 128-channel block
   (the light block gets its own shorter least-squares fit baked into
   its fold column).
 - masking: the additive -1e6 mask enters the scores PSUM via a rank-1
   matmul (ones[1,Tq]^T @ cmask[1,Tk]) that opens each batch's PSUM
   accumulation group, so Exp reads scores straight from PSUM (with
   accum_out row sums) and no DVE masking ops exist.  All-masked rows
   (valid_len==0) get e += 1 / sums += Tk fixes only when such rows
   exist in the sharded inputs.
 - tail: Exp -> PE transpose of e (fp16) -> eT -> attn@V (fp16) ->
   output copy scaled by the reciprocal row sums -> one DMA per batch.
 - fp16 inputs (queries/keys/weights/values/identity): halves input DMA
   and runs the projections at 1 cycle/row instead of fp32's 4.

Sharding: data-parallel over batch, 2 batches per core on 8 cores.
"""

import math
import sys

import numpy as np

sys.path.insert(0, "/opt/trn_rl_repo")

B, TQ, TK, DIN, H, DV = 16, 128, 256, 64, 256, 256
NCORES = 8
NB = B // NCORES
HB = 2  # h blocks of 128 partitions
NEG = -1000000.0
NEG16 = -30000.0  # f16-representable; exp underflows to 0 through f32 PSUM
PI = math.pi
FX = 65536  # fixed-point phase resolution

# int16 DVE ALU ops saturate on this HW (verified): phases stay int32 and
# ACT reads the low 16 bits via a bitcast stride-2 view (wrap = mod 2pi).
FOLD_POOL = True  # place alpha_m*wv folds on the Pool engine
# light |wv| block harmonic count (heavy block gets all M):
#   2 -> trunc m2 (1.61e-2 exact), 1 -> trunc m1+m2 (1.66e-2 at L=4.4)
LIGHT_M = 1
LADDER_M2 = True  # km2 via one exact-parent DVE Chebyshev step (off ACT)

# (fit range R, sine base half-period L, number of harmonics M)
# L > R so the fundamental phase fits 16 bits without wrapping.
FIT_LADDER = [
    (4.3, 5.9, 5),
    (5.0, 6.6, 6),
    (6.0, 7.6, 7),
    (7.5, 9.3, 9),
    (9.5, 11.6, 11),
    (12.0, 14.6, 14),
    (16.0, 19.3, 18),
]


def _fit_sine(R, L, M):
    """Least-squares fit tanh(x) ~= sum_m alpha_m sin((m+1) pi x / L) on [-R, R]."""
    x = np.linspace(-R, R, 20001)
    t = np.tanh(x)
    w = 0.05 + np.exp(-0.5 * (x / 0.6) ** 2)
    A = np.stack([np.sin((m + 1) * np.pi * x / L) for m in range(M)], axis=1)
    ATA = (A * w[:, None]).T @ A + 1e-9 * np.eye(M)
    alpha = np.linalg.solve(ATA, (A * w[:, None]).T @ t)
    return alpha.astype(np.float64)


def build_program(L, M, exp_shift=0.0, zero_vl=False, k_act=None, trunc=(),
                  stride2=False, sched4=False):
    """Build the per-core Bass program.

    k_act: 0-based harmonic indices whose k-side trig runs on ACT (the
    rest use the bf16 DVE Chebyshev ladder).  Index 0 is always ACT.
    trunc: suffix set of harmonic indices evaluated on only the first 128
    h-channels (the host sorts channels by |wv| descending, so these are
    the heavy ones; the dropped tail contributes O(alpha_m * sum|wv_lo|)).
    """
    import concourse.bacc as bacc
    import concourse.bass as bass
    import concourse.mybir as mybir
    import concourse.tile as tile

    f32 = mybir.dt.float32
    f16 = mybir.dt.float16
    i32 = mybir.dt.int32
    i16 = mybir.dt.int16
    bf16 = mybir.dt.bfloat16
    AF = mybir.ActivationFunctionType
    ALU = mybir.AluOpType

    if k_act is None:
        k_act = set(range(M))
    k_act = set(k_act) | {0}
    trunc = set(trunc)
    assert all(m + 1 in trunc or m + 1 == M for m in trunc), "trunc must be a suffix"

    def hbs(m):
        return 1 if m in trunc else HB

    E_Q = HB * NB * TQ  # 512 q elems per partition per quadrature
    E_K = HB * NB * TK  # 1024

    idt = i32
    w16_1 = float(FX / (2.0 * L))
    sin_scale = 2.0 * PI / FX

    def act_view(nphase):
        """int16 low-half view of an int32 phase tile for ACT Sin input."""
        return nphase.bitcast(i16)[..., 0::2]

    def bcast(ap_obj, reps, inner):
        """[P, reps, inner] AP: `inner` contiguous elems repeated `reps`
        times via a stride-0 middle dim."""
        return bass.AP(
            tensor=ap_obj.tensor,
            offset=ap_obj.offset,
            ap=[list(ap_obj.ap[0]), [0, reps], [1, inner]],
        )

    nc = bacc.Bacc("TRN2", target_bir_lowering=False, debug=False)

    DVP = DV + 1  # values plus a ones column: row sums fall out of attn@V
    PQ = 2 * H + NB * TQ  # wq | wk | qT   (fp16)
    PK = NB * TK  # kT   (fp16)
    PV0 = NB * 2 * DVP
    PV = PV0 + NB * TK  # vals|ones | additive mask rows (partition 0)  (fp16)
    PC = HB * M + NB  # fold[hb,m] | u  (fp32)
    packq_d = nc.dram_tensor("packq", [DIN, PQ], f16, kind="ExternalInput").ap()
    packk_d = nc.dram_tensor("packk", [DIN, PK], f16, kind="ExternalInput").ap()
    packv_d = nc.dram_tensor("packv", [128, PV], f16, kind="ExternalInput").ap()
    packc_d = nc.dram_tensor("packc", [128, PC], f32, kind="ExternalInput").ap()
    out_d = nc.dram_tensor("out", [NB, TQ, DV], f32, kind="ExternalOutput").ap()

    with tile.TileContext(nc) as tc:
        with (
            tc.tile_pool(name="singles", bufs=1) as singles,
            tc.tile_pool(name="trig", bufs=5 if stride2 else 4) as trig,
            tc.tile_pool(name="soft", bufs=2) as soft,
            tc.tile_pool(name="pproj", bufs=1, space="PSUM") as pproj,
            tc.tile_pool(name="pscore", bufs=2, space="PSUM") as pscore,
            tc.tile_pool(name="ptail", bufs=2, space="PSUM") as ptail,
        ):
            # ---- inputs (order = HWDGE trigger order) ----
            pkq = singles.tile([DIN, PQ], f16)
            nc.sync.dma_start(out=pkq, in_=packq_d)
            pkk = singles.tile([DIN, PK], f16)
            nc.sync.dma_start(out=pkk, in_=packk_d)
            pkc = singles.tile([128, PC], f32)
            nc.sync.dma_start(out=pkc, in_=packc_d)
            pkv = singles.tile([128, PV], f16)
            nc.sync.dma_start(out=pkv, in_=packv_d)

            wq_sb = pkq[:, 0:H]
            wk_sb = pkq[:, H : 2 * H]
            qTs = pkq[:, 2 * H :]
            kTs = pkk.rearrange("p (b x) -> p b x", b=NB)
            vs = pkv[:, 0:PV0].rearrange("p (b c v) -> p b c v", b=NB, c=2)
            # additive mask rows (partition 0): rank-1 lhsT per (b, k-block)
            cmask = pkv[0:1, PV0:].rearrange("p (b k) -> p b k", b=NB)
            fold_sb = pkc[:, 0 : HB * M].rearrange("p (hb m) -> p hb m", hb=HB)
            u_sb = pkc[:, HB * M : HB * M + NB]

            # ---- projections (PE, fp16 -> PSUM fp32) ----
            # qT lives in one bank per hb so the q phase (and with it the
            # whole ACT spine) starts as soon as hb0's group closes; the
            # warmup matmul borrows hb0's bank (its group is re-opened by
            # the real projection).
            warm_c = singles.tile([128, 1], f32)
            nc.vector.memset(warm_c, 0.0)
            qT_ps = [
                pproj.tile([128, NB, TQ], f32, tag=f"qT{hb}", name=f"qT{hb}")
                for hb in range(HB)
            ]
            nc.tensor.matmul(
                qT_ps[0][0:1, 0, 0:1], lhsT=warm_c, rhs=warm_c, start=True,
                stop=True,
            )
            for hb in range(HB):
                nc.tensor.matmul(
                    qT_ps[hb].rearrange("p b x -> p (b x)"),
                    lhsT=wq_sb[:, hb * 128 : (hb + 1) * 128],
                    rhs=qTs,
                    start=True,
                    stop=True,
                )
            kT_ps = pproj.tile([128, HB, NB, TK], f32)
            for hb in range(HB):
                for b in range(NB):
                    nc.tensor.matmul(
                        kT_ps[:, hb, b],
                        lhsT=wk_sb[:, hb * 128 : (hb + 1) * 128],
                        rhs=kTs[:, b],
                        start=(b == 0),
                        stop=(b == NB - 1),
                    )

            # transposed scores: scoresT[k, q] per (batch, k-block of 128).
            # Each batch's [128, 2, TQ] tile is one PSUM bank: the first
            # mask matmul opens it (start zeroes the whole bank), the last
            # harmonic matmul closes it.  The rank-1 mask matmuls (f16
            # cmask^T @ ones) run as soon as packv lands, long before the
            # harmonics, and let Exp read a whole batch in one op.
            ones1 = singles.tile([1, TQ], f16)
            nc.vector.memset(ones1, 1.0)
            scores_ps = [
                pscore.tile([128, 2, TQ], f32, tag="scores", name=f"scores{b}")
                for b in range(NB)
            ]
            n_mm = 2 + 2 * sum(2 * (1 if m in trunc else HB) for m in range(M))
            mm_i = [0] * NB
            for b in range(NB):
                for c in range(2):
                    mm_i[b] += 1
                    nc.tensor.matmul(
                        scores_ps[b][:, c],
                        lhsT=cmask[:, b, c * 128 : (c + 1) * 128],
                        rhs=ones1,
                        start=(mm_i[b] == 1),
                        stop=False,
                    )

            # ---- m0 phases: [2(sin|cos), hb, b, x], q first ----
            # cos offsets run on the (idle) Pool engine, split per hb so the
            # ACT cos Sin halves chain without stalling on the slow Pool op
            n16q = singles.tile([128, 2, HB, NB, TQ], idt)
            for hb in range(HB):
                nc.vector.tensor_scalar(
                    out=n16q[:, 0, hb], in0=qT_ps[hb], scalar1=w16_1,
                    scalar2=None, op0=ALU.mult,
                )
            for hb in range(HB):
                nc.gpsimd.tensor_scalar(
                    out=n16q[:, 1, hb], in0=n16q[:, 0, hb], scalar1=16384.0,
                    scalar2=None, op0=ALU.add,
                )
            tq1 = singles.tile([128, 2, HB, NB, TQ], bf16)
            for a in range(2):
                for hb in range(HB):
                    nc.scalar.activation(
                        out=tq1[:, a, hb], in_=act_view(n16q[:, a, hb]),
                        func=AF.Sin, bias=0.0, scale=sin_scale,
                    )
            tq_tiles = {0: tq1}

            n16k = singles.tile([128, 2, HB, NB, TK], idt)
            nc.vector.tensor_scalar(
                out=n16k[:, 0], in0=kT_ps, scalar1=w16_1, scalar2=None, op0=ALU.mult
            )
            # cos offsets: hb0 on Pool, hb1 on DVE (right after the k sin
            # phase) so the ACT cos Sin can run as one monolithic op
            nc.gpsimd.tensor_scalar(
                out=n16k[:, 1, 0], in0=n16k[:, 0, 0], scalar1=16384.0,
                scalar2=None, op0=ALU.add,
            )
            if sched4:
                nc.vector.tensor_scalar(
                    out=n16k[:, 1, 1], in0=n16k[:, 0, 1], scalar1=16384.0,
                    scalar2=None, op0=ALU.add,
                )
            else:
                nc.gpsimd.tensor_scalar(
                    out=n16k[:, 1, 1], in0=n16k[:, 0, 1], scalar1=16384.0,
                    scalar2=None, op0=ALU.add,
                )
            n1k_sin = n16k[:, 0].rearrange("p hb b x -> p (hb b x)")
            nk2 = None
            tk_tiles = {}
            tk1 = trig.tile([128, 2, HB, NB, TK], bf16, tag="tk", name="tk0")
            nc.scalar.activation(
                out=tk1[:, 0], in_=act_view(n16k[:, 0]), func=AF.Sin, bias=0.0,
                scale=sin_scale,
            )
            if sched4:
                nc.scalar.activation(
                    out=tk1[:, 1], in_=act_view(n16k[:, 1]), func=AF.Sin,
                    bias=0.0, scale=sin_scale,
                )
            else:
                for hb in range(HB):
                    nc.scalar.activation(
                        out=tk1[:, 1, hb], in_=act_view(n16k[:, 1, hb]),
                        func=AF.Sin, bias=0.0, scale=sin_scale,
                    )
            tk_tiles[0] = tk1

            # doubled cosines: Chebyshev multiplier 2*c1, quad-broadcast.
            # dq1's op is emitted lazily so the schedule controls its DVE slot.
            dq1 = singles.tile([128, E_Q], bf16)

            def emit_dq1():
                nc.vector.tensor_scalar(
                    out=dq1, in0=tq1[:, 1].rearrange("p a b x -> p (a b x)"),
                    scalar1=2.0, scalar2=None, op0=ALU.mult,
                )

            # doubled k cosine: only needed when a step-1 k ladder exists
            dk1 = None
            if any(m not in k_act and not (stride2 and m >= 3) for m in range(1, M)):
                dk1 = singles.tile([128, E_K], bf16)
                nc.vector.tensor_scalar(
                    out=dk1, in0=tk1[:, 1].rearrange("p a b x -> p (a b x)"),
                    scalar1=2.0, scalar2=None, op0=ALU.mult,
                )

            def flat(t):
                return t.rearrange("p a hb b x -> p (a hb b x)")

            def emit_fold(m, eng=None):
                # Pool for early harmonics (frees the saturated DVE stream);
                # DVE for the last one (it gates the exp tail; Pool is slow).
                if eng is None:
                    eng = nc.gpsimd if FOLD_POOL else nc.vector
                AC = trig.tile([128, 2, HB, NB, TQ], bf16, tag="AC", name=f"AC{m}")
                for hb in range(hbs(m)):
                    eng.tensor_scalar(
                        out=AC[:, :, hb], in0=tq_tiles[m][:, :, hb],
                        scalar1=fold_sb[:, hb, m : m + 1], scalar2=None, op0=ALU.mult,
                    )
                return AC

            AC_tiles = {0: emit_fold(0)}

            def emit_q_ladder(m):
                nh = hbs(m)
                tmp = trig.tile([128, 2, HB, NB, TQ], bf16, tag="qtmp", name=f"qt{m}")
                c1 = bass.AP(
                    tensor=dq1.tensor, offset=dq1.offset,
                    ap=[list(dq1.ap[0]), [0, 2], [NB * TQ, nh], [TQ, NB], [1, TQ]],
                )
                nc.vector.tensor_tensor(
                    out=tmp[:, :, 0:nh], in0=tq_tiles[m - 1][:, :, 0:nh],
                    in1=c1, op=ALU.mult,
                )
                if m == 1:
                    # t2_sin = 2 c1 s1 (s0 = 0) is tmp itself; t2_cos needs
                    # the -1, applied in place (elementwise) -> no copy
                    nc.vector.tensor_scalar(
                        out=tmp[:, 1], in0=tmp[:, 1], scalar1=-1.0, scalar2=None,
                        op0=ALU.add,
                    )
                    tq_tiles[m] = tmp
                    return
                new = trig.tile([128, 2, HB, NB, TQ], bf16, tag="qlad", name=f"ql{m}")
                nc.vector.tensor_tensor(
                    out=new[:, :, 0:nh], in0=tmp[:, :, 0:nh],
                    in1=tq_tiles[m - 2][:, :, 0:nh], op=ALU.subtract,
                )
                tq_tiles[m] = new
                tq_tiles.pop(m - 2, None)

            def emit_k_phase_pair(m, pre=None):
                """n_m = (m+1)*n1 (+16384 on the cos half): two 2x int32 ops.
                pre: tile whose cos half was already computed elsewhere."""
                nh = hbs(m)
                n_new = pre if pre is not None else trig.tile(
                    [128, 2, HB, NB, TK], idt, tag="nk", name=f"nk{m}"
                )
                src = n16k[:, 0, 0:nh].rearrange("p hb b x -> p (hb b x)")
                nc.vector.tensor_scalar(
                    out=n_new[:, 0, 0:nh].rearrange("p hb b x -> p (hb b x)"),
                    in0=src, scalar1=float(m + 1), scalar2=None, op0=ALU.mult,
                )
                if pre is None:
                    nc.vector.tensor_scalar(
                        out=n_new[:, 1, 0:nh].rearrange("p hb b x -> p (hb b x)"),
                        in0=src, scalar1=float(m + 1), scalar2=16384.0,
                        op0=ALU.mult, op1=ALU.add,
                    )
                return n_new

            def emit_k_act(m, nphase, split=False):
                nh = hbs(m)
                tk = trig.tile([128, 2, HB, NB, TK], bf16, tag="tk", name=f"tk{m}")
                if split:
                    # per (quadrature, hb) piece: downstream consumers (dk2,
                    # per-hb score matmuls) unblock as each piece lands
                    for a in range(2):
                        for hb in range(nh):
                            nc.scalar.activation(
                                out=tk[:, a, hb], in_=act_view(nphase[:, a, hb]),
                                func=AF.Sin, bias=0.0, scale=sin_scale,
                            )
                else:
                    nc.scalar.activation(
                        out=tk[:, :, 0:nh], in_=act_view(nphase[:, :, 0:nh]),
                        func=AF.Sin, bias=0.0, scale=sin_scale,
                    )
                tk_tiles[m] = tk

            def emit_k_ladder(m, step=1, dtile=None):
                """tk_m from tk_{m-step} via the angle-addition recurrence
                with increment (step)*omega_1:
                  t_{m} = 2 cos(step w1 x) (.) t_{m-step} - t_{m-2*step}
                (the second parent is the [0|1] pair when m == 2*step - 1).
                dtile: the doubled-cos multiplier (defaults to dk1)."""
                nh = hbs(m)
                if dtile is None:
                    dtile = dk1
                tmp = trig.tile([128, 2, HB, NB, TK], bf16, tag="ktmp", name=f"kt{m}")
                c1 = bass.AP(
                    tensor=dtile.tensor, offset=dtile.offset,
                    ap=[list(dtile.ap[0]), [0, 2], [NB * TK, nh], [TK, NB], [1, TK]],
                )
                nc.vector.tensor_tensor(
                    out=tmp[:, :, 0:nh], in0=tk_tiles[m - step][:, :, 0:nh],
                    in1=c1, op=ALU.mult,
                )
                if m == 2 * step - 1:
                    # sin half is tmp itself (t_{2s-1}sin = 2 c_s s_s - 0);
                    # the cos half gets its -1 in place -> no copy
                    nc.vector.tensor_scalar(
                        out=tmp[:, 1, 0:nh], in0=tmp[:, 1, 0:nh], scalar1=-1.0,
                        scalar2=None, op0=ALU.add,
                    )
                    tk_tiles[m] = tmp
                    return
                new = trig.tile([128, 2, HB, NB, TK], bf16, tag="tk", name=f"tk{m}")
                nc.vector.tensor_tensor(
                    out=new[:, :, 0:nh], in0=tmp[:, :, 0:nh],
                    in1=tk_tiles[m - 2 * step][:, :, 0:nh], op=ALU.subtract,
                )
                tk_tiles[m] = new

            def emit_scores(m):
                AC = AC_tiles.pop(m)
                tk = tk_tiles[m]
                for b in range(NB):
                    for c in range(2):
                        for hb in range(hbs(m)):
                            for qa, ka in ((0, 1), (1, 0)):
                                mm_i[b] += 1
                                nc.tensor.matmul(
                                    scores_ps[b][:, c],
                                    lhsT=tk[:, ka, hb, b, c * 128 : (c + 1) * 128],
                                    rhs=AC[:, qa, hb, b],
                                    start=False,
                                    stop=(mm_i[b] == n_mm),
                                )
                if not stride2:
                    tk_tiles.pop(m - 2, None)

            # ---- harmonic loop ----
            def emit_dk2():
                # 2*cos(2 w1 x), hb0 only (users are truncated)
                d = singles.tile([128, NB * TK], bf16)
                nc.vector.tensor_scalar(
                    out=d,
                    in0=tk_tiles[1][:, 1, 0].rearrange("p b x -> p (b x)"),
                    scalar1=2.0, scalar2=None, op0=ALU.mult,
                )
                return d

            if sched4:
                # M=3/M=4 schedule, tuned so every engine's in-order queue
                # feeds the next consumer just in time:
                #  - ACT spine: m0 sins, km1, km2, then the Exp-table load
                #    overlaps the trailing score matmuls.
                #  - DVE: k phases first (they gate the ACT spine), the
                #    q-ladder between them, the m3 coda (dk2/kt3/fold3,
                #    M=4 only) last.
                #  - Pool: cos offsets + folds m0..m2 only.
                assert M in (3, 4)
                ladder_m2 = LADDER_M2 and M == 3 and hbs(2) == 1
                merged_k12 = (
                    not ladder_m2 and M == 3 and hbs(1) == 1 and hbs(2) == 1
                )
                if ladder_m2:
                    # km1 split per quadrature half; kt2's exact-parent
                    # Chebyshev step then streams per half right behind it
                    emit_k_act(1, emit_k_phase_pair(1), split=True)
                elif merged_k12:
                    # km1+km2 share one contiguous phase tile so the two
                    # truncated harmonics cost a single ACT Sin op
                    nk12 = singles.tile([128, 2, 2, NB, TK], idt)
                    src = n16k[:, 0, 0:1].rearrange("p hb b x -> p (hb b x)")
                    for mi, mh in enumerate((1, 2)):
                        nc.vector.tensor_scalar(
                            out=nk12[:, mi, 0].rearrange("p b x -> p (b x)"),
                            in0=src, scalar1=float(mh + 1), scalar2=None,
                            op0=ALU.mult,
                        )
                        nc.vector.tensor_scalar(
                            out=nk12[:, mi, 1].rearrange("p b x -> p (b x)"),
                            in0=src, scalar1=float(mh + 1), scalar2=16384.0,
                            op0=ALU.mult, op1=ALU.add,
                        )
                    tk12 = trig.tile([128, 2, 2, NB, TK], bf16, tag="tk",
                                     name="tk12")
                    nc.scalar.activation(
                        out=tk12, in_=act_view(nk12), func=AF.Sin, bias=0.0,
                        scale=sin_scale,
                    )
                    # [m, quad, b, x] views shaped like [quad, hb=1, b, x]
                    tk_tiles[1] = tk12[:, 0:1].rearrange("p m a b x -> p a m b x")
                    tk_tiles[2] = tk12[:, 1:2].rearrange("p m a b x -> p a m b x")
                else:
                    emit_k_act(1, emit_k_phase_pair(1))
                emit_dq1()
                emit_q_ladder(1)
                AC_tiles[1] = emit_fold(1)
                emit_scores(0)
                # m2: q-side first; fold m2 split hb0->Pool / hb1->DVE so
                # neither saturated queue gates m2's score matmuls
                if not merged_k12 and not ladder_m2:
                    emit_k_act(2, emit_k_phase_pair(2))
                emit_q_ladder(2)
                if ladder_m2:
                    # km2 off the ACT spine: one exact-parent Chebyshev step
                    # (t2 = 2 cos(w) t1 - t0) on the idle DVE coda, hb0 only,
                    # emitted per quadrature half so each piece runs as soon
                    # as its km1 half lands.  The Exp table load then starts
                    # right after km1's Sin.
                    dk1h = singles.tile([128, NB * TK], bf16)
                    nc.vector.tensor_scalar(
                        out=dk1h, in0=tk_tiles[0][:, 1, 0].rearrange(
                            "p b x -> p (b x)"
                        ),
                        scalar1=2.0, scalar2=None, op0=ALU.mult,
                    )
                    c1q2 = bass.AP(
                        tensor=dk1h.tensor, offset=dk1h.offset,
                        ap=[list(dk1h.ap[0]), [TK, NB], [1, TK]],
                    )
                    kt2tmp = trig.tile(
                        [128, 2, HB, NB, TK], bf16, tag="ktmp", name="kt2"
                    )
                    kt2 = trig.tile(
                        [128, 2, HB, NB, TK], bf16, tag="tk", name="tk2"
                    )
                    for a in range(2):
                        nc.vector.tensor_tensor(
                            out=kt2tmp[:, a, 0], in0=tk_tiles[1][:, a, 0],
                            in1=c1q2, op=ALU.mult,
                        )
                        nc.vector.tensor_tensor(
                            out=kt2[:, a, 0], in0=kt2tmp[:, a, 0],
                            in1=tk_tiles[0][:, a, 0], op=ALU.subtract,
                        )
                    tk_tiles[2] = kt2
                AC2 = trig.tile([128, 2, HB, NB, TQ], bf16, tag="AC", name="AC2")
                nc.gpsimd.tensor_scalar(
                    out=AC2[:, :, 0], in0=tq_tiles[2][:, :, 0],
                    scalar1=fold_sb[:, 0, 2:3], scalar2=None, op0=ALU.mult,
                )
                if hbs(2) == 2:
                    nc.vector.tensor_scalar(
                        out=AC2[:, :, 1], in0=tq_tiles[2][:, :, 1],
                        scalar1=fold_sb[:, 1, 2:3], scalar2=None, op0=ALU.mult,
                    )
                AC_tiles[2] = AC2
                emit_scores(1)
                if M == 4:
                    # m3 coda off km1: step-2 ladder, fold on DVE
                    dk2 = emit_dk2()
                    emit_k_ladder(3, step=2, dtile=dk2)
                    emit_q_ladder(3)
                    AC_tiles[3] = emit_fold(3, eng=nc.vector)
                    emit_scores(2)
                    emit_scores(3)
                else:
                    emit_scores(2)
            else:
                emit_dq1()
                dk2 = None
                pending = 0
                for m in range(1, M):
                    if m in k_act:
                        # ACT k-harmonic: phases first (they feed the ACT
                        # spine)
                        emit_k_act(
                            m, emit_k_phase_pair(m, pre=nk2 if m == 2 else None)
                        )
                        emit_q_ladder(m)
                        AC_tiles[m] = emit_fold(m)
                    else:
                        # laddered k-harmonic: its k parents land late (off
                        # the ACT spine), so emit the ready-earlier q-side
                        # first to avoid head-blocking the in-order DVE queue
                        emit_q_ladder(m)
                        AC_tiles[m] = emit_fold(m)
                        if stride2 and m >= 3:
                            if dk2 is None:
                                dk2 = emit_dk2()
                            emit_k_ladder(m, step=2, dtile=dk2)
                        else:
                            emit_k_ladder(m)
                    emit_scores(pending)
                    pending = m
                emit_scores(pending)

            # ---- softmax + attn @ values, per batch ----
            # e^T blocks come straight off the transposed-scores PSUM; the
            # ones column appended to V turns the attn@V matmul's column DV
            # into the softmax row sums, so there is no transpose and no
            # accumulate pass.
            out_sb = soft.tile([128, NB, DV], f32, tag="out_sb", name="out_sb")
            if exp_shift > 0.0:
                bias_exp = singles.tile([128, 1], f32)
                nc.vector.memset(bias_exp, -float(exp_shift))
            # b1 first: its whole chain (Exp -> attn@V -> scale -> DMA)
            # leads, with the scale on ACT and its out-DMA on the idle Pool
            # queue (SWDGE) so the two batches' epilogues fully overlap.
            for i, b in enumerate((1, 0)):
                e = soft.tile([128, 2, TQ], f16, tag="e", name=f"e{b}")
                out_ps = ptail.tile([128, DVP], f32, tag="out_ps", name=f"out_ps{b}")
                nc.scalar.activation(
                    out=e, in_=scores_ps[b], func=AF.Exp,
                    bias=bias_exp if exp_shift > 0.0 else 0.0, scale=1.0,
                )
                if zero_vl:
                    # all-masked batches: e == 0 everywhere; add u (1 for
                    # such batches, else 0) -> uniform attention; the ones
                    # column then sums to Tk automatically.
                    nc.vector.tensor_scalar(
                        out=e, in0=e,
                        scalar1=u_sb[:, b : b + 1], scalar2=None, op0=ALU.add,
                    )
                for c in range(2):
                    nc.tensor.matmul(
                        out_ps,
                        lhsT=e[:, c],
                        rhs=vs[:, b, c],
                        start=(c == 0),
                        stop=(c == 1),
                    )
                r = soft.tile([128, 1], f32, tag="r", name=f"r{b}")
                nc.vector.reciprocal(out=r, in_=out_ps[:, DV:DVP])
                nc.vector.tensor_scalar(
                    out=out_sb[:, b], in0=out_ps[:, 0:DV], scalar1=r,
                    scalar2=None, op0=ALU.mult,
                )
                if i == 0:
                    nc.gpsimd.dma_start(out=out_d[b], in_=out_sb[:, b])
                else:
                    nc.sync.dma_start(out=out_d[b], in_=out_sb[:, b])

    nc.compile()
    return nc


def prepare_in_maps(queries, keys, values, valid_lens, wq, wk, wv, alpha,
                    alpha_light=None, exp_shift=0.0):
    """alpha_light: fold coefficients for the hb1 (light-|wv|) block — a
    dedicated shorter fit when the trailing harmonics are truncated."""
    M = len(alpha)
    if alpha_light is None:
        alpha_light = alpha
    queries = np.asarray(queries, dtype=np.float32)
    keys = np.asarray(keys, dtype=np.float32)
    values = np.asarray(values, dtype=np.float32)
    wq = np.asarray(wq, dtype=np.float32)
    wk = np.asarray(wk, dtype=np.float32)
    wv = np.asarray(wv, dtype=np.float32)
    valid_lens = np.asarray(valid_lens)

    # sort h-channels by |wv| descending (exact permutation of the h-sum)
    # so truncated harmonics keep the heavy channels in hb block 0
    order = np.argsort(-np.abs(wv), kind="stable")
    wq = wq[:, order]
    wk = wk[:, order]
    wv = wv[order]

    # fold[p, hb, m] = alpha_m * wv[hb*128 + p]; hb1 uses its own fit
    al = np.zeros(M, np.float64)
    al[: len(alpha_light)] = np.asarray(alpha_light, np.float64)
    fold = np.empty((128, HB, M), np.float32)
    for hb, a in ((0, np.asarray(alpha, np.float64)), (1, al)):
        fold[:, hb, :] = (
            a[None, :] * wv[hb * 128 : (hb + 1) * 128, None]
        ).astype(np.float32)
    ar = np.arange(TK)
    in_maps = []
    for c in range(NCORES):
        bs = slice(c * NB, (c + 1) * NB)
        qT = queries[bs].transpose(2, 0, 1).reshape(DIN, NB * TQ)
        kT = keys[bs].transpose(2, 0, 1).reshape(DIN, NB * TK)
        packq = np.concatenate([wq, wk, qT], axis=1).astype(np.float16)
        packk = kT.astype(np.float16)
        # vals with a ones column appended per (b, k-block): attn@V's last
        # output column becomes the softmax row sum.  The additive mask
        # rides along as f16 rows on partition 0 (-30000 saturates exp to
        # 0 through the f32 PSUM accumulation).
        vals = values[bs].reshape(NB, 2, 128, DV).transpose(2, 0, 1, 3)
        valsp = np.concatenate(
            [vals, np.ones((128, NB, 2, 1), vals.dtype)], axis=3
        )
        u = np.zeros((128, NB), np.float32)
        maskrow = np.zeros((128, NB * TK), np.float32)
        for j, vl in enumerate(valid_lens[bs]):
            vl = int(vl)
            if vl <= 0:
                maskrow[0, j * TK : (j + 1) * TK] = NEG16
                u[:, j] = 1.0
            else:
                maskrow[0, j * TK : (j + 1) * TK] = np.where(ar < vl, 0.0, NEG16)
        packv = np.concatenate(
            [valsp.reshape(128, NB * 2 * (DV + 1)), maskrow], axis=1
        ).astype(np.float16)
        packc = np.concatenate(
            [fold.reshape(128, HB * M), u], axis=1
        ).astype(np.float32)
        in_maps.append(
            {
                "packq": np.ascontiguousarray(packq),
                "packk": np.ascontiguousarray(packk),
                "packv": np.ascontiguousarray(packv),
                "packc": np.ascontiguousarray(packc),
            }
        )
    return in_maps


def _pick_fit(queries, keys, wq, wk):
    q = queries.reshape(-1, DIN).astype(np.float32) @ wq.astype(np.float32)
    k = keys.reshape(-1, DIN).astype(np.float32) @ wk.astype(np.float32)
    qb = q.reshape(B, TQ, H)
    kb = k.reshape(B, TK, H)
    hi = (qb.max(1) + kb.max(1)).max()
    lo = (qb.min(1) + kb.min(1)).min()
    # Softmax only needs fit accuracy near the score maxima: scores far
    # below `hi` get negligible attention weight, so the fit range R only
    # has to cover the positive side (validated to 6.5e-3 output rel err
    # for this regime with lo down to -4.8).
    if hi <= 3.55 and lo >= -4.8:
        return (3.6, 4.4, 3) if LIGHT_M == 1 else (3.6, 4.6, 3)
    r_needed = max(abs(hi), abs(lo))
    for R, L, M in FIT_LADDER:
        if R >= r_needed + 0.05:
            return R, L, M
    return FIT_LADDER[-1]


_prog_cache = {}


def kernel(queries, keys, values, valid_lens, wq, wk, wv):
    from concourse import bass_utils

    queries = np.asarray(queries)
    keys = np.asarray(keys)
    values = np.asarray(values)
    valid_lens = np.asarray(valid_lens)
    wq = np.asarray(wq)
    wk = np.asarray(wk)
    wv = np.asarray(wv)

    R, L, M = _pick_fit(queries, keys, wq, wk)
    alpha = _fit_sine(R, L, M)
    bound = float(np.abs(wv).sum()) * 1.01
    exp_shift = max(0.0, bound - 60.0)
    zero_vl = bool((np.asarray(valid_lens) <= 0).any())

    # Channel truncation of the smallest-alpha harmonics is validated per
    # fit (output rel err vs the 2e-2 gate); laddered trailing harmonics
    # keep the Exp table load off the ACT critical path.
    sched4 = False
    if M == 3:
        trunc = tuple(range(LIGHT_M, 3))
        k_act = (0, 1, 2)
        stride2 = False
        sched4 = True
    elif M == 4:
        trunc = (3,)
        k_act = (0, 1, 2)
        stride2 = True
        sched4 = True
    elif M == 5:
        trunc = (M - 2, M - 1)
        k_act = (0, 1, 2)
        stride2 = True
    else:
        trunc = ()
        k_act = None
        stride2 = False

    key = (R, L, M, round(exp_shift, 3), zero_vl, trunc, k_act)
    if key not in _prog_cache:
        _prog_cache[key] = build_program(
            L, M, exp_shift, zero_vl=zero_vl, k_act=k_act, trunc=trunc,
            stride2=stride2, sched4=sched4,
        )
    nc = _prog_cache[key]

    alpha_light = _fit_sine(R, L, M - len(trunc)) if trunc else None
    in_maps = prepare_in_maps(
        queries, keys, values, valid_lens, wq, wk, wv, alpha, alpha_light,
        exp_shift=exp_shift,
    )
    res = bass_utils.run_bass_kernel_spmd(nc, in_maps, core_ids=list(range(NCORES)))
    out = np.concatenate([r["out"] for r in res.results], axis=0)
    return out.astype(np.float32)


if __name__ == "__main__":
    rng = np.random.default_rng(0)
    inputs = {
        "queries": rng.standard_normal((B, TQ, DIN), dtype=np.float32),
        "keys": rng.standard_normal((B, TK, DIN), dtype=np.float32),
        "values": rng.standard_normal((B, TK, DV), dtype=np.float32),
        "valid_lens": rng.integers(0, TK, size=(B,)).astype(np.int32),
        "wq": (rng.standard_normal((DIN, H), dtype=np.float32) * 0.05),
        "wk": (rng.standard_normal((DIN, H), dtype=np.float32) * 0.05),
        "wv": (rng.standard_normal((H,), dtype=np.float32) * 0.05),
    }
    out = kernel(**inputs)
    print("out", out.shape, out.dtype)



# revision 69
# speedup vs baseline: 1.0122x; 1.0122x over previous
"""Bahdanau attention Trainium2 kernel, v2.

Math: out = softmax_k(mask(score)) @ values with
  score[b,q,k] = sum_h wv[h] * tanh(Q[b,q,h] + K[b,k,h]),
  Q = queries @ wq, K = keys @ wk.

tanh(x) ~= sum_m alpha_m sin(omega_m x), omega_m = (m+1) pi / L; the
angle-addition identity factorizes tanh(q+k) into 2M dense [Tq,H]x[H,Tk]
matmuls.  v2 balances trig generation across all five engines:

 - Fixed-point phases (x * 65536/(2L) as int32; ACT reads the low 16
   bits via a bitcast stride-2 view = phase mod 2pi; int16 DVE ALU ops
   saturate on this HW so the arithmetic stays int32).  The fundamental
   comes off the projection PSUM with one convert per side; harmonic m's
   phases are (m+1)*n1 (+16384 quarter-turn for cos), two 2x DVE ops;
   the two m0 cos offsets run on the otherwise-idle Pool engine.
 - k-side trig: ACT Sin per harmonic for the low harmonics; the two
   truncated top harmonics come from one stride-2 Chebyshev step each
   (sin((j+2)w x) = 2 cos(2 w x) sin(j w x) - sin((j-2) w x)) on DVE,
   with exact ACT-computed parents, so the Exp table reload starts two
   Sins earlier.
 - q-side trig: ACT Sin for the fundamental only, then a bf16 Chebyshev
   ladder on DVE (2 tensor_tensor ops per harmonic, 2x mode; the 2*cos
   multiplier is read with a stride-0 quadrature-broadcast AP).
 - alpha_m * wv fold: per-partition-pointer tensor_scalar on the smaller
   q side; Pool engine for all but the exp-gating last harmonic.  The
   host sorts h-channels by |wv| descending (an exact permutation), and
   the two smallest-alpha harmonics use only the heavy 128-channel block
   (the light block gets its own shorter least-squares fit baked into
   its fold column).
 - masking: the additive -1e6 mask enters the scores PSUM via a rank-1
   matmul (ones[1,Tq]^T @ cmask[1,Tk]) that opens each batch's PSUM
   accumulation group, so Exp reads scores straight from PSUM (with
   accum_out row sums) and no DVE masking ops exist.  All-masked rows
   (valid_len==0) get e += 1 / sums += Tk fixes only when such rows
   exist in the sharded inputs.
 - tail: Exp -> PE transpose of e (fp16) -> eT -> attn@V (fp16) ->
   output copy scaled by the reciprocal row sums -> one DMA per batch.
 - fp16 inputs (queries/keys/weights/values/identity): halves input DMA
   and runs the projections at 1 cycle/row instead of fp32's 4.

Sharding: data-parallel over batch, 2 batches per core on 8 cores.
"""

import math
import sys

import numpy as np

sys.path.insert(0, "/opt/trn_rl_repo")

B, TQ, TK, DIN, H, DV = 16, 128, 256, 64, 256, 256
NCORES = 8
NB = B // NCORES
HB = 2  # h blocks of 128 partitions
NEG = -1000000.0
NEG16 = -30000.0  # f16-representable; exp underflows to 0 through f32 PSUM
PI = math.pi
FX = 65536  # fixed-point phase resolution

# int16 DVE ALU ops saturate on this HW (verified): phases stay int32 and
# ACT reads the low 16 bits via a bitcast stride-2 view (wrap = mod 2pi).
FOLD_POOL = True  # place alpha_m*wv folds on the Pool engine
# light |wv| block harmonic count (heavy block gets all M):
#   2 -> trunc m2 (1.61e-2 exact), 1 -> trunc m1+m2 (1.66e-2 at L=4.4)
LIGHT_M = 1
LADDER_M2 = False  # km2 via one exact-parent DVE Chebyshev step (off ACT)
LADDER_M1 = True  # km1 via the 2-op special-case ladder; km2 stays on ACT

# (fit range R, sine base half-period L, number of harmonics M)
# L > R so the fundamental phase fits 16 bits without wrapping.
FIT_LADDER = [
    (4.3, 5.9, 5),
    (5.0, 6.6, 6),
    (6.0, 7.6, 7),
    (7.5, 9.3, 9),
    (9.5, 11.6, 11),
    (12.0, 14.6, 14),
    (16.0, 19.3, 18),
]


def _fit_sine(R, L, M):
    """Least-squares fit tanh(x) ~= sum_m alpha_m sin((m+1) pi x / L) on [-R, R]."""
    x = np.linspace(-R, R, 20001)
    t = np.tanh(x)
    w = 0.05 + np.exp(-0.5 * (x / 0.6) ** 2)
    A = np.stack([np.sin((m + 1) * np.pi * x / L) for m in range(M)], axis=1)
    ATA = (A * w[:, None]).T @ A + 1e-9 * np.eye(M)
    alpha = np.linalg.solve(ATA, (A * w[:, None]).T @ t)
    return alpha.astype(np.float64)


def build_program(L, M, exp_shift=0.0, zero_vl=False, k_act=None, trunc=(),
                  stride2=False, sched4=False):
    """Build the per-core Bass program.

    k_act: 0-based harmonic indices whose k-side trig runs on ACT (the
    rest use the bf16 DVE Chebyshev ladder).  Index 0 is always ACT.
    trunc: suffix set of harmonic indices evaluated on only the first 128
    h-channels (the host sorts channels by |wv| descending, so these are
    the heavy ones; the dropped tail contributes O(alpha_m * sum|wv_lo|)).
    """
    import concourse.bacc as bacc
    import concourse.bass as bass
    import concourse.mybir as mybir
    import concourse.tile as tile

    f32 = mybir.dt.float32
    f16 = mybir.dt.float16
    i32 = mybir.dt.int32
    i16 = mybir.dt.int16
    bf16 = mybir.dt.bfloat16
    AF = mybir.ActivationFunctionType
    ALU = mybir.AluOpType

    if k_act is None:
        k_act = set(range(M))
    k_act = set(k_act) | {0}
    trunc = set(trunc)
    assert all(m + 1 in trunc or m + 1 == M for m in trunc), "trunc must be a suffix"

    def hbs(m):
        return 1 if m in trunc else HB

    E_Q = HB * NB * TQ  # 512 q elems per partition per quadrature
    E_K = HB * NB * TK  # 1024

    idt = i32
    w16_1 = float(FX / (2.0 * L))
    sin_scale = 2.0 * PI / FX

    def act_view(nphase):
        """int16 low-half view of an int32 phase tile for ACT Sin input."""
        return nphase.bitcast(i16)[..., 0::2]

    def bcast(ap_obj, reps, inner):
        """[P, reps, inner] AP: `inner` contiguous elems repeated `reps`
        times via a stride-0 middle dim."""
        return bass.AP(
            tensor=ap_obj.tensor,
            offset=ap_obj.offset,
            ap=[list(ap_obj.ap[0]), [0, reps], [1, inner]],
        )

    nc = bacc.Bacc("TRN2", target_bir_lowering=False, debug=False)

    DVP = DV + 1  # values plus a ones column: row sums fall out of attn@V
    PQ = 2 * H + NB * TQ  # wq | wk | qT   (fp16)
    PK = NB * TK  # kT   (fp16)
    PV0 = NB * 2 * DVP
    PV = PV0 + NB * TK  # vals|ones | additive mask rows (partition 0)  (fp16)
    PC = HB * M + NB  # fold[hb,m] | u  (fp32)
    packq_d = nc.dram_tensor("packq", [DIN, PQ], f16, kind="ExternalInput").ap()
    packk_d = nc.dram_tensor("packk", [DIN, PK], f16, kind="ExternalInput").ap()
    packv_d = nc.dram_tensor("packv", [128, PV], f16, kind="ExternalInput").ap()
    packc_d = nc.dram_tensor("packc", [128, PC], f32, kind="ExternalInput").ap()
    out_d = nc.dram_tensor("out", [NB, TQ, DV], f32, kind="ExternalOutput").ap()

    with tile.TileContext(nc) as tc:
        with (
            tc.tile_pool(name="singles", bufs=1) as singles,
            tc.tile_pool(name="trig", bufs=5 if stride2 else 4) as trig,
            tc.tile_pool(name="soft", bufs=2) as soft,
            tc.tile_pool(name="pproj", bufs=1, space="PSUM") as pproj,
            tc.tile_pool(name="pscore", bufs=2, space="PSUM") as pscore,
            tc.tile_pool(name="ptail", bufs=2, space="PSUM") as ptail,
        ):
            # ---- inputs (order = HWDGE trigger order) ----
            pkq = singles.tile([DIN, PQ], f16)
            nc.sync.dma_start(out=pkq, in_=packq_d)
            pkk = singles.tile([DIN, PK], f16)
            nc.sync.dma_start(out=pkk, in_=packk_d)
            pkc = singles.tile([128, PC], f32)
            nc.sync.dma_start(out=pkc, in_=packc_d)
            pkv = singles.tile([128, PV], f16)
            nc.sync.dma_start(out=pkv, in_=packv_d)

            wq_sb = pkq[:, 0:H]
            wk_sb = pkq[:, H : 2 * H]
            qTs = pkq[:, 2 * H :]
            kTs = pkk.rearrange("p (b x) -> p b x", b=NB)
            vs = pkv[:, 0:PV0].rearrange("p (b c v) -> p b c v", b=NB, c=2)
            # additive mask rows (partition 0): rank-1 lhsT per (b, k-block)
            cmask = pkv[0:1, PV0:].rearrange("p (b k) -> p b k", b=NB)
            fold_sb = pkc[:, 0 : HB * M].rearrange("p (hb m) -> p hb m", hb=HB)
            u_sb = pkc[:, HB * M : HB * M + NB]

            # ---- projections (PE, fp16 -> PSUM fp32) ----
            # qT lives in one bank per hb so the q phase (and with it the
            # whole ACT spine) starts as soon as hb0's group closes; the
            # warmup matmul borrows hb0's bank (its group is re-opened by
            # the real projection).
            warm_c = singles.tile([128, 1], f32)
            nc.vector.memset(warm_c, 0.0)
            qT_ps = [
                pproj.tile([128, NB, TQ], f32, tag=f"qT{hb}", name=f"qT{hb}")
                for hb in range(HB)
            ]
            nc.tensor.matmul(
                qT_ps[0][0:1, 0, 0:1], lhsT=warm_c, rhs=warm_c, start=True,
                stop=True,
            )
            for hb in range(HB):
                nc.tensor.matmul(
                    qT_ps[hb].rearrange("p b x -> p (b x)"),
                    lhsT=wq_sb[:, hb * 128 : (hb + 1) * 128],
                    rhs=qTs,
                    start=True,
                    stop=True,
                )
            kT_ps = pproj.tile([128, HB, NB, TK], f32)
            for hb in range(HB):
                for b in range(NB):
                    nc.tensor.matmul(
                        kT_ps[:, hb, b],
                        lhsT=wk_sb[:, hb * 128 : (hb + 1) * 128],
                        rhs=kTs[:, b],
                        start=(b == 0),
                        stop=(b == NB - 1),
                    )

            # transposed scores: scoresT[k, q] per (batch, k-block of 128).
            # Each batch's [128, 2, TQ] tile is one PSUM bank: the first
            # mask matmul opens it (start zeroes the whole bank), the last
            # harmonic matmul closes it.  The rank-1 mask matmuls (f16
            # cmask^T @ ones) run as soon as packv lands, long before the
            # harmonics, and let Exp read a whole batch in one op.
            ones1 = singles.tile([1, TQ], f16)
            nc.vector.memset(ones1, 1.0)
            scores_ps = [
                pscore.tile([128, 2, TQ], f32, tag="scores", name=f"scores{b}")
                for b in range(NB)
            ]
            n_mm = 2 + 2 * sum(2 * (1 if m in trunc else HB) for m in range(M))
            mm_i = [0] * NB
            for b in range(NB):
                for c in range(2):
                    mm_i[b] += 1
                    nc.tensor.matmul(
                        scores_ps[b][:, c],
                        lhsT=cmask[:, b, c * 128 : (c + 1) * 128],
                        rhs=ones1,
                        start=(mm_i[b] == 1),
                        stop=False,
                    )

            # ---- m0 phases: [2(sin|cos), hb, b, x], q first ----
            # cos offsets run on the (idle) Pool engine, split per hb so the
            # ACT cos Sin halves chain without stalling on the slow Pool op
            n16q = singles.tile([128, 2, HB, NB, TQ], idt)
            for hb in range(HB):
                nc.vector.tensor_scalar(
                    out=n16q[:, 0, hb], in0=qT_ps[hb], scalar1=w16_1,
                    scalar2=None, op0=ALU.mult,
                )
            for hb in range(HB):
                nc.gpsimd.tensor_scalar(
                    out=n16q[:, 1, hb], in0=n16q[:, 0, hb], scalar1=16384.0,
                    scalar2=None, op0=ALU.add,
                )
            tq1 = singles.tile([128, 2, HB, NB, TQ], bf16)
            for a in range(2):
                for hb in range(HB):
                    nc.scalar.activation(
                        out=tq1[:, a, hb], in_=act_view(n16q[:, a, hb]),
                        func=AF.Sin, bias=0.0, scale=sin_scale,
                    )
            tq_tiles = {0: tq1}

            n16k = singles.tile([128, 2, HB, NB, TK], idt)
            nc.vector.tensor_scalar(
                out=n16k[:, 0], in0=kT_ps, scalar1=w16_1, scalar2=None, op0=ALU.mult
            )
            # cos offsets: hb0 on Pool, hb1 on DVE (right after the k sin
            # phase) so the ACT cos Sin can run as one monolithic op
            nc.gpsimd.tensor_scalar(
                out=n16k[:, 1, 0], in0=n16k[:, 0, 0], scalar1=16384.0,
                scalar2=None, op0=ALU.add,
            )
            if sched4:
                nc.vector.tensor_scalar(
                    out=n16k[:, 1, 1], in0=n16k[:, 0, 1], scalar1=16384.0,
                    scalar2=None, op0=ALU.add,
                )
            else:
                nc.gpsimd.tensor_scalar(
                    out=n16k[:, 1, 1], in0=n16k[:, 0, 1], scalar1=16384.0,
                    scalar2=None, op0=ALU.add,
                )
            n1k_sin = n16k[:, 0].rearrange("p hb b x -> p (hb b x)")
            nk2 = None
            tk_tiles = {}
            tk1 = trig.tile([128, 2, HB, NB, TK], bf16, tag="tk", name="tk0")
            nc.scalar.activation(
                out=tk1[:, 0], in_=act_view(n16k[:, 0]), func=AF.Sin, bias=0.0,
                scale=sin_scale,
            )
            if sched4:
                nc.scalar.activation(
                    out=tk1[:, 1], in_=act_view(n16k[:, 1]), func=AF.Sin,
                    bias=0.0, scale=sin_scale,
                )
            else:
                for hb in range(HB):
                    nc.scalar.activation(
                        out=tk1[:, 1, hb], in_=act_view(n16k[:, 1, hb]),
                        func=AF.Sin, bias=0.0, scale=sin_scale,
                    )
            tk_tiles[0] = tk1

            # doubled cosines: Chebyshev multiplier 2*c1, quad-broadcast.
            # dq1's op is emitted lazily so the schedule controls its DVE slot.
            dq1 = singles.tile([128, E_Q], bf16)

            def emit_dq1():
                nc.vector.tensor_scalar(
                    out=dq1, in0=tq1[:, 1].rearrange("p a b x -> p (a b x)"),
                    scalar1=2.0, scalar2=None, op0=ALU.mult,
                )

            # doubled k cosine: only needed when a step-1 k ladder exists
            dk1 = None
            if any(m not in k_act and not (stride2 and m >= 3) for m in range(1, M)):
                dk1 = singles.tile([128, E_K], bf16)
                nc.vector.tensor_scalar(
                    out=dk1, in0=tk1[:, 1].rearrange("p a b x -> p (a b x)"),
                    scalar1=2.0, scalar2=None, op0=ALU.mult,
                )

            def flat(t):
                return t.rearrange("p a hb b x -> p (a hb b x)")

            def emit_fold(m, eng=None):
                # Pool for early harmonics (frees the saturated DVE stream);
                # DVE for the last one (it gates the exp tail; Pool is slow).
                if eng is None:
                    eng = nc.gpsimd if FOLD_POOL else nc.vector
                AC = trig.tile([128, 2, HB, NB, TQ], bf16, tag="AC", name=f"AC{m}")
                for hb in range(hbs(m)):
                    eng.tensor_scalar(
                        out=AC[:, :, hb], in0=tq_tiles[m][:, :, hb],
                        scalar1=fold_sb[:, hb, m : m + 1], scalar2=None, op0=ALU.mult,
                    )
                return AC

            AC_tiles = {0: emit_fold(0)}

            def emit_q_ladder(m):
                nh = hbs(m)
                tmp = trig.tile([128, 2, HB, NB, TQ], bf16, tag="qtmp", name=f"qt{m}")
                c1 = bass.AP(
                    tensor=dq1.tensor, offset=dq1.offset,
                    ap=[list(dq1.ap[0]), [0, 2], [NB * TQ, nh], [TQ, NB], [1, TQ]],
                )
                nc.vector.tensor_tensor(
                    out=tmp[:, :, 0:nh], in0=tq_tiles[m - 1][:, :, 0:nh],
                    in1=c1, op=ALU.mult,
                )
                if m == 1:
                    # t2_sin = 2 c1 s1 (s0 = 0) is tmp itself; t2_cos needs
                    # the -1, applied in place (elementwise) -> no copy
                    nc.vector.tensor_scalar(
                        out=tmp[:, 1], in0=tmp[:, 1], scalar1=-1.0, scalar2=None,
                        op0=ALU.add,
                    )
                    tq_tiles[m] = tmp
                    return
                new = trig.tile([128, 2, HB, NB, TQ], bf16, tag="qlad", name=f"ql{m}")
                nc.vector.tensor_tensor(
                    out=new[:, :, 0:nh], in0=tmp[:, :, 0:nh],
                    in1=tq_tiles[m - 2][:, :, 0:nh], op=ALU.subtract,
                )
                tq_tiles[m] = new
                tq_tiles.pop(m - 2, None)

            def emit_k_phase_pair(m, pre=None):
                """n_m = (m+1)*n1 (+16384 on the cos half): two 2x int32 ops.
                pre: tile whose cos half was already computed elsewhere."""
                nh = hbs(m)
                n_new = pre if pre is not None else trig.tile(
                    [128, 2, HB, NB, TK], idt, tag="nk", name=f"nk{m}"
                )
                src = n16k[:, 0, 0:nh].rearrange("p hb b x -> p (hb b x)")
                nc.vector.tensor_scalar(
                    out=n_new[:, 0, 0:nh].rearrange("p hb b x -> p (hb b x)"),
                    in0=src, scalar1=float(m + 1), scalar2=None, op0=ALU.mult,
                )
                if pre is None:
                    nc.vector.tensor_scalar(
                        out=n_new[:, 1, 0:nh].rearrange("p hb b x -> p (hb b x)"),
                        in0=src, scalar1=float(m + 1), scalar2=16384.0,
                        op0=ALU.mult, op1=ALU.add,
                    )
                return n_new

            def emit_k_act(m, nphase, split=False):
                nh = hbs(m)
                tk = trig.tile([128, 2, HB, NB, TK], bf16, tag="tk", name=f"tk{m}")
                if split:
                    # per (quadrature, hb) piece: downstream consumers (dk2,
                    # per-hb score matmuls) unblock as each piece lands
                    for a in range(2):
                        for hb in range(nh):
                            nc.scalar.activation(
                                out=tk[:, a, hb], in_=act_view(nphase[:, a, hb]),
                                func=AF.Sin, bias=0.0, scale=sin_scale,
                            )
                else:
                    nc.scalar.activation(
                        out=tk[:, :, 0:nh], in_=act_view(nphase[:, :, 0:nh]),
                        func=AF.Sin, bias=0.0, scale=sin_scale,
                    )
                tk_tiles[m] = tk

            def emit_k_ladder(m, step=1, dtile=None):
                """tk_m from tk_{m-step} via the angle-addition recurrence
                with increment (step)*omega_1:
                  t_{m} = 2 cos(step w1 x) (.) t_{m-step} - t_{m-2*step}
                (the second parent is the [0|1] pair when m == 2*step - 1).
                dtile: the doubled-cos multiplier (defaults to dk1)."""
                nh = hbs(m)
                if dtile is None:
                    dtile = dk1
                tmp = trig.tile([128, 2, HB, NB, TK], bf16, tag="ktmp", name=f"kt{m}")
                c1 = bass.AP(
                    tensor=dtile.tensor, offset=dtile.offset,
                    ap=[list(dtile.ap[0]), [0, 2], [NB * TK, nh], [TK, NB], [1, TK]],
                )
                nc.vector.tensor_tensor(
                    out=tmp[:, :, 0:nh], in0=tk_tiles[m - step][:, :, 0:nh],
                    in1=c1, op=ALU.mult,
                )
                if m == 2 * step - 1:
                    # sin half is tmp itself (t_{2s-1}sin = 2 c_s s_s - 0);
                    # the cos half gets its -1 in place -> no copy
                    nc.vector.tensor_scalar(
                        out=tmp[:, 1, 0:nh], in0=tmp[:, 1, 0:nh], scalar1=-1.0,
                        scalar2=None, op0=ALU.add,
                    )
                    tk_tiles[m] = tmp
                    return
                new = trig.tile([128, 2, HB, NB, TK], bf16, tag="tk", name=f"tk{m}")
                nc.vector.tensor_tensor(
                    out=new[:, :, 0:nh], in0=tmp[:, :, 0:nh],
                    in1=tk_tiles[m - 2 * step][:, :, 0:nh], op=ALU.subtract,
                )
                tk_tiles[m] = new

            def emit_scores(m):
                AC = AC_tiles.pop(m)
                tk = tk_tiles[m]
                for b in range(NB):
                    for c in range(2):
                        for hb in range(hbs(m)):
                            for qa, ka in ((0, 1), (1, 0)):
                                mm_i[b] += 1
                                nc.tensor.matmul(
                                    scores_ps[b][:, c],
                                    lhsT=tk[:, ka, hb, b, c * 128 : (c + 1) * 128],
                                    rhs=AC[:, qa, hb, b],
                                    start=False,
                                    stop=(mm_i[b] == n_mm),
                                )
                if not stride2:
                    tk_tiles.pop(m - 2, None)

            # ---- harmonic loop ----
            def emit_dk2():
                # 2*cos(2 w1 x), hb0 only (users are truncated)
                d = singles.tile([128, NB * TK], bf16)
                nc.vector.tensor_scalar(
                    out=d,
                    in0=tk_tiles[1][:, 1, 0].rearrange("p b x -> p (b x)"),
                    scalar1=2.0, scalar2=None, op0=ALU.mult,
                )
                return d

            if sched4:
                # M=3/M=4 schedule, tuned so every engine's in-order queue
                # feeds the next consumer just in time:
                #  - ACT spine: m0 sins, km1, km2, then the Exp-table load
                #    overlaps the trailing score matmuls.
                #  - DVE: k phases first (they gate the ACT spine), the
                #    q-ladder between them, the m3 coda (dk2/kt3/fold3,
                #    M=4 only) last.
                #  - Pool: cos offsets + folds m0..m2 only.
                assert M in (3, 4)
                ladder_m1 = LADDER_M1 and M == 3 and hbs(1) == 1
                ladder_m2 = (
                    not ladder_m1 and LADDER_M2 and M == 3 and hbs(2) == 1
                )
                merged_k12 = (
                    not ladder_m1 and not ladder_m2
                    and M == 3 and hbs(1) == 1 and hbs(2) == 1
                )
                if ladder_m1:
                    # km1 off the ACT spine via the 2-op special-case ladder
                    # (t1 = 2 cos(w) t0 - [0|1]); km2 stays on ACT so its
                    # score matmuls fire first while kt1 finishes on DVE.
                    emit_k_act(2, emit_k_phase_pair(2))
                    emit_dq1()
                    emit_q_ladder(1)
                    AC_tiles[1] = emit_fold(1)
                    emit_q_ladder(2)
                    AC2 = trig.tile(
                        [128, 2, HB, NB, TQ], bf16, tag="AC", name="AC2"
                    )
                    nc.gpsimd.tensor_scalar(
                        out=AC2[:, :, 0], in0=tq_tiles[2][:, :, 0],
                        scalar1=fold_sb[:, 0, 2:3], scalar2=None, op0=ALU.mult,
                    )
                    if hbs(2) == 2:
                        nc.vector.tensor_scalar(
                            out=AC2[:, :, 1], in0=tq_tiles[2][:, :, 1],
                            scalar1=fold_sb[:, 1, 2:3], scalar2=None,
                            op0=ALU.mult,
                        )
                    AC_tiles[2] = AC2
                    dk1h = singles.tile([128, NB * TK], bf16)
                    nc.vector.tensor_scalar(
                        out=dk1h, in0=tk_tiles[0][:, 1, 0].rearrange(
                            "p b x -> p (b x)"
                        ),
                        scalar1=2.0, scalar2=None, op0=ALU.mult,
                    )
                    emit_k_ladder(1, step=1, dtile=dk1h)
                    emit_scores(0)
                    emit_scores(2)
                    emit_scores(1)
                elif ladder_m2:
                    # km1 split per quadrature half; kt2's exact-parent
                    # Chebyshev step then streams per half right behind it
                    emit_k_act(1, emit_k_phase_pair(1), split=True)
                elif merged_k12:
                    # km1+km2 share one contiguous phase tile so the two
                    # truncated harmonics cost a single ACT Sin op
                    nk12 = singles.tile([128, 2, 2, NB, TK], idt)
                    src = n16k[:, 0, 0:1].rearrange("p hb b x -> p (hb b x)")
                    for mi, mh in enumerate((1, 2)):
                        nc.vector.tensor_scalar(
                            out=nk12[:, mi, 0].rearrange("p b x -> p (b x)"),
                            in0=src, scalar1=float(mh + 1), scalar2=None,
                            op0=ALU.mult,
                        )
                        nc.vector.tensor_scalar(
                            out=nk12[:, mi, 1].rearrange("p b x -> p (b x)"),
                            in0=src, scalar1=float(mh + 1), scalar2=16384.0,
                            op0=ALU.mult, op1=ALU.add,
                        )
                    tk12 = trig.tile([128, 2, 2, NB, TK], bf16, tag="tk",
                                     name="tk12")
                    nc.scalar.activation(
                        out=tk12, in_=act_view(nk12), func=AF.Sin, bias=0.0,
                        scale=sin_scale,
                    )
                    # [m, quad, b, x] views shaped like [quad, hb=1, b, x]
                    tk_tiles[1] = tk12[:, 0:1].rearrange("p m a b x -> p a m b x")
                    tk_tiles[2] = tk12[:, 1:2].rearrange("p m a b x -> p a m b x")
                else:
                    emit_k_act(1, emit_k_phase_pair(1))
                if not ladder_m1:
                    emit_dq1()
                    emit_q_ladder(1)
                    AC_tiles[1] = emit_fold(1)
                    emit_scores(0)
                    # m2: q-side first; fold m2 split hb0->Pool / hb1->DVE
                    # so neither saturated queue gates m2's score matmuls
                    if not merged_k12 and not ladder_m2:
                        emit_k_act(2, emit_k_phase_pair(2))
                    emit_q_ladder(2)
                if ladder_m2:
                    # km2 off the ACT spine: one exact-parent Chebyshev step
                    # (t2 = 2 cos(w) t1 - t0) on the idle DVE coda, hb0 only,
                    # emitted per quadrature half so each piece runs as soon
                    # as its km1 half lands.  The Exp table load then starts
                    # right after km1's Sin.
                    dk1h = singles.tile([128, NB * TK], bf16)
                    nc.vector.tensor_scalar(
                        out=dk1h, in0=tk_tiles[0][:, 1, 0].rearrange(
                            "p b x -> p (b x)"
                        ),
                        scalar1=2.0, scalar2=None, op0=ALU.mult,
                    )
                    c1q2 = bass.AP(
                        tensor=dk1h.tensor, offset=dk1h.offset,
                        ap=[list(dk1h.ap[0]), [TK, NB], [1, TK]],
                    )
                    kt2tmp = trig.tile(
                        [128, 2, HB, NB, TK], bf16, tag="ktmp", name="kt2"
                    )
                    kt2 = trig.tile(
                        [128, 2, HB, NB, TK], bf16, tag="tk", name="tk2"
                    )
                    for a in range(2):
                        nc.vector.tensor_tensor(
                            out=kt2tmp[:, a, 0], in0=tk_tiles[1][:, a, 0],
                            in1=c1q2, op=ALU.mult,
                        )
                        nc.vector.tensor_tensor(
                            out=kt2[:, a, 0], in0=kt2tmp[:, a, 0],
                            in1=tk_tiles[0][:, a, 0], op=ALU.subtract,
                        )
                    tk_tiles[2] = kt2
                if not ladder_m1:
                    AC2 = trig.tile(
                        [128, 2, HB, NB, TQ], bf16, tag="AC", name="AC2"
                    )
                    nc.gpsimd.tensor_scalar(
                        out=AC2[:, :, 0], in0=tq_tiles[2][:, :, 0],
                        scalar1=fold_sb[:, 0, 2:3], scalar2=None, op0=ALU.mult,
                    )
                    if hbs(2) == 2:
                        nc.vector.tensor_scalar(
                            out=AC2[:, :, 1], in0=tq_tiles[2][:, :, 1],
                            scalar1=fold_sb[:, 1, 2:3], scalar2=None,
                            op0=ALU.mult,
                        )
                    AC_tiles[2] = AC2
                    emit_scores(1)
                    if M == 4:
                        # m3 coda off km1: step-2 ladder, fold on DVE
                        dk2 = emit_dk2()
                        emit_k_ladder(3, step=2, dtile=dk2)
                        emit_q_ladder(3)
                        AC_tiles[3] = emit_fold(3, eng=nc.vector)
                        emit_scores(2)
                        emit_scores(3)
                    else:
                        emit_scores(2)
            else:
                emit_dq1()
                dk2 = None
                pending = 0
                for m in range(1, M):
                    if m in k_act:
                        # ACT k-harmonic: phases first (they feed the ACT
                        # spine)
                        emit_k_act(
                            m, emit_k_phase_pair(m, pre=nk2 if m == 2 else None)
                        )
                        emit_q_ladder(m)
                        AC_tiles[m] = emit_fold(m)
                    else:
                        # laddered k-harmonic: its k parents land late (off
                        # the ACT spine), so emit the ready-earlier q-side
                        # first to avoid head-blocking the in-order DVE queue
                        emit_q_ladder(m)
                        AC_tiles[m] = emit_fold(m)
                        if stride2 and m >= 3:
                            if dk2 is None:
                                dk2 = emit_dk2()
                            emit_k_ladder(m, step=2, dtile=dk2)
                        else:
                            emit_k_ladder(m)
                    emit_scores(pending)
                    pending = m
                emit_scores(pending)

            # ---- softmax + attn @ values, per batch ----
            # e^T blocks come straight off the transposed-scores PSUM; the
            # ones column appended to V turns the attn@V matmul's column DV
            # into the softmax row sums, so there is no transpose and no
            # accumulate pass.
            out_sb = soft.tile([128, NB, DV], f32, tag="out_sb", name="out_sb")
            if exp_shift > 0.0:
                bias_exp = singles.tile([128, 1], f32)
                nc.vector.memset(bias_exp, -float(exp_shift))
            # b1 first: its whole chain (Exp -> attn@V -> scale -> DMA)
            # leads, with the scale on ACT and its out-DMA on the idle Pool
            # queue (SWDGE) so the two batches' epilogues fully overlap.
            for i, b in enumerate((1, 0)):
                e = soft.tile([128, 2, TQ], f16, tag="e", name=f"e{b}")
                out_ps = ptail.tile([128, DVP], f32, tag="out_ps", name=f"out_ps{b}")
                nc.scalar.activation(
                    out=e, in_=scores_ps[b], func=AF.Exp,
                    bias=bias_exp if exp_shift > 0.0 else 0.0, scale=1.0,
                )
                if zero_vl:
                    # all-masked batches: e == 0 everywhere; add u (1 for
                    # such batches, else 0) -> uniform attention; the ones
                    # column then sums to Tk automatically.
                    nc.vector.tensor_scalar(
                        out=e, in0=e,
                        scalar1=u_sb[:, b : b + 1], scalar2=None, op0=ALU.add,
                    )
                for c in range(2):
                    nc.tensor.matmul(
                        out_ps,
                        lhsT=e[:, c],
                        rhs=vs[:, b, c],
                        start=(c == 0),
                        stop=(c == 1),
                    )
                r = soft.tile([128, 1], f32, tag="r", name=f"r{b}")
                nc.vector.reciprocal(out=r, in_=out_ps[:, DV:DVP])
                nc.vector.tensor_scalar(
                    out=out_sb[:, b], in0=out_ps[:, 0:DV], scalar1=r,
                    scalar2=None, op0=ALU.mult,
                )
                if i == 0:
                    nc.gpsimd.dma_start(out=out_d[b], in_=out_sb[:, b])
                else:
                    nc.sync.dma_start(out=out_d[b], in_=out_sb[:, b])

    nc.compile()
    return nc


def prepare_in_maps(queries, keys, values, valid_lens, wq, wk, wv, alpha,
                    alpha_light=None, exp_shift=0.0):
    """alpha_light: fold coefficients for the hb1 (light-|wv|) block — a
    dedicated shorter fit when the trailing harmonics are truncated."""
    M = len(alpha)
    if alpha_light is None:
        alpha_light = alpha
    queries = np.asarray(queries, dtype=np.float32)
    keys = np.asarray(keys, dtype=np.float32)
    values = np.asarray(values, dtype=np.float32)
    wq = np.asarray(wq, dtype=np.float32)
    wk = np.asarray(wk, dtype=np.float32)
    wv = np.asarray(wv, dtype=np.float32)
    valid_lens = np.asarray(valid_lens)

    # sort h-channels by |wv| descending (exact permutation of the h-sum)
    # so truncated harmonics keep the heavy channels in hb block 0
    order = np.argsort(-np.abs(wv), kind="stable")
    wq = wq[:, order]
    wk = wk[:, order]
    wv = wv[order]

    # fold[p, hb, m] = alpha_m * wv[hb*128 + p]; hb1 uses its own fit
    al = np.zeros(M, np.float64)
    al[: len(alpha_light)] = np.asarray(alpha_light, np.float64)
    fold = np.empty((128, HB, M), np.float32)
    for hb, a in ((0, np.asarray(alpha, np.float64)), (1, al)):
        fold[:, hb, :] = (
            a[None, :] * wv[hb * 128 : (hb + 1) * 128, None]
        ).astype(np.float32)
    ar = np.arange(TK)
    in_maps = []
    for c in range(NCORES):
        bs = slice(c * NB, (c + 1) * NB)
        qT = queries[bs].transpose(2, 0, 1).reshape(DIN, NB * TQ)
        kT = keys[bs].transpose(2, 0, 1).reshape(DIN, NB * TK)
        packq = np.concatenate([wq, wk, qT], axis=1).astype(np.float16)
        packk = kT.astype(np.float16)
        # vals with a ones column appended per (b, k-block): attn@V's last
        # output column becomes the softmax row sum.  The additive mask
        # rides along as f16 rows on partition 0 (-30000 saturates exp to
        # 0 through the f32 PSUM accumulation).
        vals = values[bs].reshape(NB, 2, 128, DV).transpose(2, 0, 1, 3)
        valsp = np.concatenate(
            [vals, np.ones((128, NB, 2, 1), vals.dtype)], axis=3
        )
        u = np.zeros((128, NB), np.float32)
        maskrow = np.zeros((128, NB * TK), np.float32)
        for j, vl in enumerate(valid_lens[bs]):
            vl = int(vl)
            if vl <= 0:
                maskrow[0, j * TK : (j + 1) * TK] = NEG16
                u[:, j] = 1.0
            else:
                maskrow[0, j * TK : (j + 1) * TK] = np.where(ar < vl, 0.0, NEG16)
        packv = np.concatenate(
            [valsp.reshape(128, NB * 2 * (DV + 1)), maskrow], axis=1
        ).astype(np.float16)
        packc = np.concatenate(
            [fold.reshape(128, HB * M), u], axis=1
        ).astype(np.float32)
        in_maps.append(
            {
                "packq": np.ascontiguousarray(packq),
                "packk": np.ascontiguousarray(packk),
                "packv": np.ascontiguousarray(packv),
                "packc": np.ascontiguousarray(packc),
            }
        )
    return in_maps


def _pick_fit(queries, keys, wq, wk):
    q = queries.reshape(-1, DIN).astype(np.float32) @ wq.astype(np.float32)
    k = keys.reshape(-1, DIN).astype(np.float32) @ wk.astype(np.float32)
    qb = q.reshape(B, TQ, H)
    kb = k.reshape(B, TK, H)
    hi = (qb.max(1) + kb.max(1)).max()
    lo = (qb.min(1) + kb.min(1)).min()
    # Softmax only needs fit accuracy near the score maxima: scores far
    # below `hi` get negligible attention weight, so the fit range R only
    # has to cover the positive side (validated to 6.5e-3 output rel err
    # for this regime with lo down to -4.8).
    if hi <= 3.55 and lo >= -4.8:
        return (3.6, 4.4, 3) if LIGHT_M == 1 else (3.6, 4.6, 3)
    r_needed = max(abs(hi), abs(lo))
    for R, L, M in FIT_LADDER:
        if R >= r_needed + 0.05:
            return R, L, M
    return FIT_LADDER[-1]


_prog_cache = {}


def kernel(queries, keys, values, valid_lens, wq, wk, wv):
    from concourse import bass_utils

    queries = np.asarray(queries)
    keys = np.asarray(keys)
    values = np.asarray(values)
    valid_lens = np.asarray(valid_lens)
    wq = np.asarray(wq)
    wk = np.asarray(wk)
    wv = np.asarray(wv)

    R, L, M = _pick_fit(queries, keys, wq, wk)
    alpha = _fit_sine(R, L, M)
    bound = float(np.abs(wv).sum()) * 1.01
    exp_shift = max(0.0, bound - 60.0)
    zero_vl = bool((np.asarray(valid_lens) <= 0).any())

    # Channel truncation of the smallest-alpha harmonics is validated per
    # fit (output rel err vs the 2e-2 gate); laddered trailing harmonics
    # keep the Exp table load off the ACT critical path.
    sched4 = False
    if M == 3:
        trunc = tuple(range(LIGHT_M, 3))
        k_act = (0, 1, 2)
        stride2 = False
        sched4 = True
    elif M == 4:
        trunc = (3,)
        k_act = (0, 1, 2)
        stride2 = True
        sched4 = True
    elif M == 5:
        trunc = (M - 2, M - 1)
        k_act = (0, 1, 2)
        stride2 = True
    else:
        trunc = ()
        k_act = None
        stride2 = False

    key = (R, L, M, round(exp_shift, 3), zero_vl, trunc, k_act)
    if key not in _prog_cache:
        _prog_cache[key] = build_program(
            L, M, exp_shift, zero_vl=zero_vl, k_act=k_act, trunc=trunc,
            stride2=stride2, sched4=sched4,
        )
    nc = _prog_cache[key]

    alpha_light = _fit_sine(R, L, M - len(trunc)) if trunc else None
    in_maps = prepare_in_maps(
        queries, keys, values, valid_lens, wq, wk, wv, alpha, alpha_light,
        exp_shift=exp_shift,
    )
    res = bass_utils.run_bass_kernel_spmd(nc, in_maps, core_ids=list(range(NCORES)))
    out = np.concatenate([r["out"] for r in res.results], axis=0)
    return out.astype(np.float32)


if __name__ == "__main__":
    rng = np.random.default_rng(0)
    inputs = {
        "queries": rng.standard_normal((B, TQ, DIN), dtype=np.float32),
        "keys": rng.standard_normal((B, TK, DIN), dtype=np.float32),
        "values": rng.standard_normal((B, TK, DV), dtype=np.float32),
        "valid_lens": rng.integers(0, TK, size=(B,)).astype(np.int32),
        "wq": (rng.standard_normal((DIN, H), dtype=np.float32) * 0.05),
        "wk": (rng.standard_normal((DIN, H), dtype=np.float32) * 0.05),
        "wv": (rng.standard_normal((H,), dtype=np.float32) * 0.05),
    }
    out = kernel(**inputs)
    print("out", out.shape, out.dtype)

